# revision 60
# baseline (speedup 1.0000x reference)
"""Trainium2 Bass kernel for GQA attention with RoPE (dense transformer).

Problem: B=2, S=2048, H=2048, 16 query heads / 4 KV heads, head_dim 128,
causal flash-style attention, fused QKV + o_proj.

Sharding (8 cores): (batch, head-group) grid. Core c handles batch c//4 and
head group c%4 (4 query heads + their shared KV head). o_proj is computed as
per-group partials reduced on host (tensor-parallel o_proj input split).

v3 vs v2 (245us): warmup trimmed to 6 (real matmuls continue the HAM clock
ramp with useful work), ones matrix via memset, q-projection chains emitted
before k/v so the A psum banks free up early, attention(0) moved to t=1,
v transposes emitted after attention, pairwise ic-outer o_proj tail with
split output DMAs.

On-core layout: activations live as [feature, token] ("transposed") so the
feature contraction dims land on SBUF partitions for the PE array.
Causal masking: fully-masked k-tiles are skipped entirely; diagonal tiles
get a zero-fill triangle (affine_select on GpSimd) after exp.
"""
import math

import numpy as np

import concourse.bass as bass
import concourse.mybir as mybir
import concourse.tile as tile
from concourse import bacc
from concourse.bass_utils import run_bass_kernel_spmd
from concourse.masks import make_identity

B, S, H = 2, 2048, 2048
NH, KVH, HD = 16, 4, 128
G = 4                 # head groups (= KVH); grid = G x B = 8 cores
GQ = NH // KVH        # query heads per group
QD = GQ * HD          # per-core q dim (512)
KC = H // 128         # contraction chunks for projections (16)
TC = 4                # token chunks of 512
TT = S // 128         # 128-token tiles (16)

F32 = mybir.dt.float32
BF = mybir.dt.bfloat16
AF = mybir.ActivationFunctionType

_NC = None


def _emit(nc):
    # All big inputs are host-packed partition-major: row p is the full
    # contiguous per-partition payload, so every DMA is 128 descriptors of
    # >=4KB regardless of logical shape (HWDGE issue cost is ~5ns/descriptor).
    # x is packed token-chunk-major: [p, t, ko, c] so chunk t's whole
    # projection input is one contiguous 16KB-per-partition run.
    xP = nc.dram_tensor("xP", [128, KC * S], BF, kind="ExternalInput").ap()
    wqP = nc.dram_tensor("wqP", [128, KC * QD], BF, kind="ExternalInput").ap()
    wkP = nc.dram_tensor("wkP", [128, KC * HD], BF, kind="ExternalInput").ap()
    wvP = nc.dram_tensor("wvP", [128, KC * HD], BF, kind="ExternalInput").ap()
    woP = nc.dram_tensor("woP", [128, GQ * H], BF, kind="ExternalInput").ap()
    cosT = nc.dram_tensor("cosT", [HD, S], BF, kind="ExternalInput").ap()
    sinS = nc.dram_tensor("sinS", [HD, S], BF, kind="ExternalInput").ap()
    bqkv = nc.dram_tensor("bqkv", [128, 6], F32, kind="ExternalInput").ap()
    outp = nc.dram_tensor("outp", [S, H], BF, kind="ExternalOutput").ap()

    with tile.TileContext(nc) as tc:
        with (
            tc.tile_pool(name="persist", bufs=1) as pp,
            tc.tile_pool(name="qfp", bufs=2) as pqf,
            tc.tile_pool(name="cd", bufs=1) as pd,
            tc.tile_pool(name="expp", bufs=1) as pe,
            tc.tile_pool(name="psum8", bufs=1, space="PSUM") as ps8,
        ):
            # persistent per-chunk K/V (split per t-chunk to keep dep ranges
            # disjoint between the producing chunk and attention readers)
            kf = [pp.tile([128, 512], BF, name=f"kf{t}") for t in range(TC)]
            v_sb = [pp.tile([128, 4, HD], BF, name=f"vsb{t}")
                    for t in range(TC)]
            ofl = pd.tile([128, GQ, S], BF)       # normalized attn outT

            # ---- constants ----
            # ones first (one cheap memset): the PE warmup matmuls run on
            # it, so they start as soon as the GpSimd queue opens instead
            # of waiting for the multi-op make_identity chain.
            ones_mat = pp.tile([128, 128], BF)
            nc.gpsimd.memset(ones_mat[:, :], 1.0)

            # PE warmup: dummy matmuls bridge the whole initial DMA window
            # (first x/wq pieces land ~5us after the queue opens), keeping
            # the HAM activity monitor fed so the ramp to 2.4GHz completes
            # before the first real matmul instead of stretching chunk 0.
            warm = ps8.tile([128, 128], F32, tag="Ct", bufs=2, name="warm")
            for _ in range(6):
                nc.tensor.matmul(warm[:, :], ones_mat[:, :], ones_mat[:, :],
                                 start=True, stop=True)

            ident = pp.tile([128, 128], BF)
            make_identity(nc, ident[:, :])

            bias_sb = pp.tile([128, 6], F32)
            nc.gpsimd.dma_start(bias_sb[:, :], bqkv)

            def jspan(qc, j):
                if j < 4 * qc:
                    q0, n = 512 * qc, 512
                else:
                    q0 = 128 * j
                    n = 512 * (qc + 1) - q0
                return q0, n, q0 - 512 * qc

            def attention(qc, qf_t, filler=None, tail_work=None):
                """flash attention for q-chunk qc over k-tiles 0..4qc+3.

                filler(h) emits PE-dense side work (o_proj tiles of the
                previous chunk) interleaved per head, so the scheduler has
                matmuls to run while exp paces the score pipeline.
                tail_work() emits independent PE work between the last
                head's attn-value chain and its softmax finish, hiding the
                final ones-matmul's wait on the DVE exp-sum chain."""
                qs = slice(512 * qc, 512 * qc + 512)
                nj = 4 * qc + 4
                state = {}

                def finish(h):
                    # softmax denominator + normalization for head h; emitted
                    # one head late so its ones-matmul never stalls PE on the
                    # DVE accumulation chain.
                    exs, p_o = state[h]
                    p_sum = ps8.tile([128, 512], F32, tag="Bt", bufs=2,
                                     name=f"psum_{h}_{qc}")
                    nc.tensor.matmul(p_sum[:, :], ones_mat[:, :], exs[:, :],
                                     start=True, stop=True)
                    bc = pe.tile([128, 512], F32, tag="bc", bufs=2,
                                 name=f"bc_{h}_{qc}")
                    nc.vector.reciprocal_approx_fast(bc[:, :], p_sum[:, :])
                    nc.vector.tensor_mul(ofl[:, h, qs], p_o[:, :], bc[:, :])

                for h in range(GQ):
                    if filler is not None:
                        filler(h)
                    exs = pe.tile([128, 512], BF, tag="exs", bufs=3,
                                  name=f"exs_{h}_{qc}")
                    exts = []
                    for j in range(nj):
                        q0, n, off = jspan(qc, j)
                        ql = q0 - 512 * qc
                        # chunk 0's 4 score tiles fit in A0/A1 (exp paces
                        # the rotation anyway); A2/A3 then carry its k/v
                        # projection filler chains
                        ps = ps8.tile([128, 512], F32,
                                      tag=f"A{j % 4 if qc else j % 2}",
                                      name=f"ps_{h}_{qc}_{j}")
                        nc.tensor.matmul(
                            ps[:, 0:n], kf[j // 4][:, 128 * (j % 4):
                                                   128 * (j % 4) + 128],
                            qf_t[:, h, ql:ql + n], start=True, stop=True)
                        ex = pe.tile([128, 512], BF, tag="E", bufs=28,
                                     name=f"ex_{h}_{qc}_{j}")
                        nc.scalar.activation(ex[:, 0:n], ps[:, 0:n], AF.Exp)
                        if j >= 4 * qc:
                            # zero the strictly-lower (q < k) triangle
                            nc.gpsimd.affine_select(
                                out=ex[:, 0:128], in_=ex[:, 0:128],
                                compare_op=mybir.AluOpType.is_ge, fill=0.0,
                                base=0, pattern=[[1, 128]],
                                channel_multiplier=-1)
                        if j == 0:
                            nc.vector.tensor_copy(exs[:, :], ex[:, :])
                        else:
                            nc.vector.tensor_add(exs[:, ql:ql + n],
                                                 exs[:, ql:ql + n],
                                                 ex[:, 0:n])
                        exts.append(ex)
                    p_o = ps8.tile([128, 512], F32, tag="Ct", bufs=2,
                                   name=f"po_{h}_{qc}")
                    state[h] = (exs, p_o)
                    for j in range(nj):
                        q0, n, off = jspan(qc, j)
                        nc.tensor.matmul(
                            p_o[:, off:off + n],
                            v_sb[j // 4][:, j % 4, :],
                            exts[j][:, 0:n], start=(j == 0), stop=(j == nj - 1))
                    if h > 0:
                        finish(h - 1)
                if tail_work is not None:
                    tail_work()
                finish(GQ - 1)

            def oproj_tile(tt, pwo, wo_sb):
                """o_proj partial for one 128-token tile (filler mode).

                Runs two waves of 2 output-column groups on the Bt psum
                slots only, so the attention pipeline keeps both Ct slots
                for its held p_o accumulators."""
                tsl = slice(128 * tt, 128 * tt + 128)
                fo = pwo.tile([128, 4, 512], BF, tag="fo", bufs=3,
                              name=f"fo_{tt}")
                oc = 0
                for w in range(2):
                    pfs = [ps8.tile([128, 512], F32, tag="Bt", bufs=2,
                                    name=f"pf_{tt}_{oc + i}")
                           for i in range(2)]
                    for ic in range(GQ):
                        for i in range(2):
                            osl = slice(512 * (oc + i), 512 * (oc + i) + 512)
                            nc.tensor.matmul(
                                pfs[i][:, :], ofl[:, ic, tsl],
                                wo_sb[:, ic, osl],
                                start=(ic == 0), stop=(ic == GQ - 1))
                    # split psum evictions between ACT and DVE
                    for i in range(2):
                        if i % 2 == 0:
                            nc.scalar.copy(fo[:, oc + i, :], pfs[i][:, :])
                        else:
                            nc.vector.tensor_copy(fo[:, oc + i, :],
                                                  pfs[i][:, :])
                    oc += 2
                nc.sync.dma_start(outp[tsl, :], fo[:, :, :])

            def oproj_tail(pwo, wo_sb):
                """Final 4 o_proj tiles, after the last attention chunk.

                Tile pairs share a 4-psum wave (A banks, idle now) with the
                head (ic) loop OUTER, so the ic<3 matmuls are not gated on
                the last head's softmax-normalize chain. Each tile's output
                ships as two half-row DMAs on alternating queues so the
                exit barrier waits only on the last 0.25MB."""
                t0 = 4 * (TC - 1)
                fos = {}
                for p in range(2):
                    pair = (t0 + 2 * p, t0 + 2 * p + 1)
                    for tt in pair:
                        fos[tt] = pwo.tile([128, 4, 512], BF, tag="fo",
                                           bufs=3, name=f"fo_{tt}")
                    for w in range(2):
                        combos = [(tt, 2 * w + g) for tt in pair
                                  for g in range(2)]
                        pfs = [ps8.tile([128, 512], F32, tag=f"A{i}",
                                        name=f"pf_{tt}_{oc}")
                               for i, (tt, oc) in enumerate(combos)]
                        for ic in range(GQ):
                            for i, (tt, oc) in enumerate(combos):
                                nc.tensor.matmul(
                                    pfs[i][:, :],
                                    ofl[:, ic, 128 * tt:128 * tt + 128],
                                    wo_sb[:, ic, 512 * oc:512 * oc + 512],
                                    start=(ic == 0), stop=(ic == GQ - 1))
                        for i, (tt, oc) in enumerate(combos):
                            if i % 2 == 0:
                                nc.scalar.copy(fos[tt][:, oc, :], pfs[i][:, :])
                            else:
                                nc.vector.tensor_copy(fos[tt][:, oc, :],
                                                      pfs[i][:, :])
                        for tt in pair:
                            eng = [nc.sync, nc.gpsimd, nc.scalar,
                                   nc.sync][tt - t0]
                            eng.dma_start(
                                outp[128 * tt:128 * tt + 128,
                                     1024 * w:1024 * w + 1024],
                                fos[tt][:, 2 * w:2 * w + 2, :])

            # ============ interleaved projections + attention =============
            qf_tiles = [None] * TC
            with (
                tc.tile_pool(name="projw", bufs=1) as pw,
                tc.tile_pool(name="rope", bufs=1) as pr,
                tc.tile_pool(name="wop", bufs=1) as pwo,
            ):
                wq_sb = pw.tile([128, KC, QD], BF)
                wk_sb = pw.tile([128, KC, HD], BF)
                wv_sb = pw.tile([128, KC, HD], BF)
                cos_sb = pw.tile([128, S], BF)
                sin_sb = pw.tile([128, S], BF)
                wo_sb = pwo.tile([128, GQ, H], BF)

                # All input DMAs on the SP HWDGE queue, in first-needed
                # order, sized so the ko=0 accumulation starts after ~1MB.
                x_sb = [None] * TC

                def ld_x(t, pieces):
                    xt = x_sb[t]
                    if xt is None:
                        xt = pw.tile([128, KC, 512], BF, tag="xc", bufs=2,
                                     name=f"x_sb{t}")
                        x_sb[t] = xt
                    for k0, k1 in pieces:
                        nc.sync.dma_start(
                            xt[:, k0:k1, :],
                            xP[:, 512 * (KC * t + k0):512 * (KC * t + k1)])

                def ld_wq(k0, k1):
                    nc.sync.dma_start(wq_sb[:, k0:k1, :],
                                      wqP[:, QD * k0:QD * k1])

                ld_x(0, [(0, 1)])
                ld_wq(0, 1)
                ld_x(0, [(1, 2)])
                ld_wq(1, 2)
                ld_x(0, [(2, 4)])
                ld_wq(2, 4)
                nc.sync.dma_start(wk_sb[:, :, :], wkP)
                nc.sync.dma_start(wv_sb[:, :, :], wvP)
                ld_x(0, [(4, 8)])
                ld_wq(4, 8)
                ld_x(0, [(8, 16)])
                ld_wq(8, 16)
                nc.sync.dma_start(cos_sb[:, :], cosT)
                nc.sync.dma_start(sin_sb[:, :], sinS)

                for t in range(TC):
                    ts = slice(512 * t, 512 * t + 512)
                    if t + 1 < TC:
                        ld_x(t + 1, [(0, 8), (8, 16)])
                    if t == 0:
                        nc.sync.dma_start(wo_sb[:, :, :], woP)

                    pq = [ps8.tile([128, 512], F32, tag=f"A{m}",
                                   name=f"pq{m}_{t}")
                          for m in range(GQ)]
                    # q first over all ko, then k, then v: the A psum banks
                    # finish (and can be evicted) while the k/v chains still
                    # run, so the attention scores below wait less.
                    for ko in range(KC):
                        st = (ko == 0)
                        sp = (ko == KC - 1)
                        xc = x_sb[t][:, ko, :]
                        for m in range(GQ):
                            nc.tensor.matmul(
                                pq[m][:, :],
                                wq_sb[:, ko, 128 * m:128 * m + 128],
                                xc, start=st, stop=sp)

                    # q psum evictions (+bias) on ACT, one by one as each
                    # head's chain retires
                    raws = []
                    for m in range(GQ):
                        raw = pr.tile([128, 512], BF, tag="raw", bufs=6,
                                      name=f"raw_{t}_{m}")
                        nc.scalar.activation(
                            raw[:, :], pq[m][:, :], AF.Identity,
                            bias=bias_sb[:, m:m + 1])
                        raws.append((m, raw))

                    vT_box = [None]

                    def emit_k(t=t, raws=raws, tag="Bt", bufs=2):
                        pk = ps8.tile([128, 512], F32, tag=tag, bufs=bufs,
                                      name=f"pk_{t}")
                        for ko in range(KC):
                            nc.tensor.matmul(
                                pk[:, :], wk_sb[:, ko, :],
                                x_sb[t][:, ko, :],
                                start=(ko == 0), stop=(ko == KC - 1))
                        rawk = pr.tile([128, 512], BF, tag="raw", bufs=6,
                                       name=f"raw_{t}_k")
                        nc.scalar.activation(rawk[:, :], pk[:, :],
                                             AF.Identity,
                                             bias=bias_sb[:, 4:5])
                        raws.append((GQ, rawk))

                    def emit_v(t=t, vT_box=vT_box, tag="Bt", bufs=2):
                        pv = ps8.tile([128, 512], F32, tag=tag, bufs=bufs,
                                      name=f"pv_{t}")
                        for ko in range(KC):
                            nc.tensor.matmul(
                                pv[:, :], wv_sb[:, ko, :],
                                x_sb[t][:, ko, :],
                                start=(ko == 0), stop=(ko == KC - 1))
                        # v: evict with bias (frees the psum bank before
                        # attention needs it for softmax denominators)
                        vT_t = pr.tile([128, 512], BF, tag="vT", bufs=2,
                                       name=f"vT_{t}")
                        nc.scalar.activation(vT_t[:, :], pv[:, :],
                                             AF.Identity,
                                             bias=bias_sb[:, 5:6])
                        vT_box[0] = vT_t

                    # attention + o_proj for the previous chunk, emitted
                    # before this chunk's v-transpose and RoPE (both only
                    # feed the NEXT iteration) so the PE never waits on
                    # their eviction chains. attention(0) has no o_proj of
                    # its own to interleave — its PE filler is this chunk's
                    # k/v chains (on the A2/A3 banks its scores don't use).
                    def emit_ptr(t=t, vT_box=vT_box, a_tags=True):
                        # v transpose to natural [tok, d] layout (needed by
                        # the NEXT iteration's attention). As attention
                        # tail-work it rides the A banks (free once the last
                        # scores are read); Ct would deadlock on the held
                        # p_o accumulator.
                        vT_t = vT_box[0]
                        for st4 in range(4):
                            ptr = ps8.tile(
                                [128, 128], BF,
                                tag=(f"A{st4}" if a_tags else "Ct"),
                                bufs=(1 if a_tags else 2),
                                name=f"ptr_{t}_{st4}")
                            nc.tensor.transpose(
                                ptr[:, :], vT_t[:, 128 * st4:128 * st4 + 128],
                                ident[:, :])
                            nc.scalar.copy(v_sb[t][:, st4, :], ptr[:, :])

                    if t == 1:
                        fill = {0: lambda: emit_k(tag="A2", bufs=1),
                                1: lambda: emit_v(tag="A3", bufs=1)}
                        attention(0, qf_tiles[0],
                                  filler=lambda h: fill.get(h, lambda: 0)(),
                                  tail_work=emit_ptr)
                    else:
                        emit_k()
                        emit_v()
                        if t >= 2:
                            qc = t - 1
                            attention(qc, qf_tiles[qc],
                                      filler=lambda h, q=qc: oproj_tile(
                                          4 * (q - 1) + h, pwo, wo_sb),
                                      tail_work=emit_ptr)
                        else:
                            emit_ptr(a_tags=False)

                    # RoPE for q/k on DVE in [d, tok] layout (needed only by
                    # THIS chunk's attention, one iteration later)
                    qf_t = pqf.tile([128, GQ, 512], BF, tag="qf",
                                    name=f"qf_{t}")
                    qf_tiles[t] = qf_t
                    for m, raw in raws:
                        rot = pr.tile([128, 512], BF, tag="rot", bufs=2,
                                      name=f"rot_{t}_{m}")
                        nc.vector.tensor_copy(rot[0:64, :], raw[64:128, :])
                        nc.vector.tensor_copy(rot[64:128, :], raw[0:64, :])
                        t1 = pr.tile([128, 512], BF, tag="t1", bufs=2,
                                     name=f"t1_{t}_{m}")
                        nc.vector.tensor_mul(t1[:, :], rot[:, :],
                                             sin_sb[:, ts])
                        t2 = pr.tile([128, 512], BF, tag="t2", bufs=2,
                                     name=f"t2_{t}_{m}")
                        nc.vector.tensor_mul(t2[:, :], raw[:, :],
                                             cos_sb[:, ts])
                        dst = (qf_t[:, m, :] if m < GQ else kf[t][:, :])
                        nc.vector.tensor_add(dst, t1[:, :], t2[:, :])
                qc = TC - 1
                attention(qc, qf_tiles[qc],
                          filler=lambda h: oproj_tile(4 * (qc - 1) + h,
                                                      pwo, wo_sb))
                oproj_tail(pwo, wo_sb)


def _build():
    global _NC
    if _NC is None:
        nc = bacc.Bacc("TRN2", target_bir_lowering=False, debug=False,
                       num_devices=8)
        _emit(nc)
        nc.compile()
        _NC = nc
    return _NC


def _prep_inputs(x, wq, bq, wk, bk, wv, bv, wo, bo, cos, sin):
    """Host-side shard + layout prep. Core c = (g, b): g = c % 4, b = c // 4."""
    import ml_dtypes
    bf16 = ml_dtypes.bfloat16
    inv_sqrt_d = 1.0 / math.sqrt(HD)
    f32 = np.float32
    cosT = np.ascontiguousarray(cos.T.astype(bf16))
    sinSf = sin.T.astype(f32).copy()
    sinSf[0:HD // 2] *= -1.0
    sinS = np.ascontiguousarray(sinSf.astype(bf16))

    def pack(mT):
        """[n*128, m] -> [128, n*m]: row p = concat_n mT[n*128+p, :]."""
        n = mT.shape[0] // 128
        return np.ascontiguousarray(
            mT.reshape(n, 128, mT.shape[1]).transpose(1, 0, 2)
            .reshape(128, n * mT.shape[1]).astype(bf16))

    def pack_x(xT):
        """[H, S] -> [128, TC*KC*512]: [p, t, ko, c] = xT[ko*128+p, 512t+c]
        (token-chunk-major so each projection chunk is one contiguous run)."""
        return np.ascontiguousarray(
            xT.reshape(KC, 128, TC, 512).transpose(1, 2, 0, 3)
            .reshape(128, TC * KC * 512).astype(bf16))

    xPb = [pack_x(x[b].T.astype(f32)) for b in range(B)]

    in_maps = []
    for c in range(8):
        g, b = c % G, c // G
        wq_s = wq[QD * g:QD * (g + 1), :] * inv_sqrt_d
        bq_s = bq[QD * g:QD * (g + 1)] * inv_sqrt_d
        wk_s = wk[HD * g:HD * (g + 1), :]
        bk_s = bk[HD * g:HD * (g + 1)]
        wv_s = wv[HD * g:HD * (g + 1), :]
        bv_s = bv[HD * g:HD * (g + 1)]
        bias = np.zeros((128, 6), f32)
        bias[:, 0:4] = bq_s.reshape(GQ, HD).T
        bias[:, 4] = bk_s
        bias[:, 5] = bv_s
        in_maps.append({
            "xP": xPb[b],
            "wqP": pack(wq_s.T),
            "wkP": pack(wk_s.T),
            "wvP": pack(wv_s.T),
            "woP": pack(wo[:, QD * g:QD * (g + 1)].T),
            "cosT": cosT,
            "sinS": sinS,
            "bqkv": bias,
        })
    return in_maps


def run(inputs, trace=False):
    """Returns (full_output, BassKernelResults)."""
    inputs = {k: np.asarray(v) for k, v in inputs.items()}
    nc = _build()
    in_maps = _prep_inputs(**inputs)
    res = run_bass_kernel_spmd(nc, in_maps, core_ids=list(range(8)),
                               trace=trace)
    bo = inputs["bo"].astype(np.float64)
    out = np.empty((B, S, H), np.float32)
    for b in range(B):
        acc = np.zeros((S, H), np.float64)
        for g in range(G):
            acc += res.results[G * b + g]["outp"].astype(np.float64)
        out[b] = (acc + bo).astype(np.float32)
    return out, res


def kernel(**inputs):
    return run(inputs, trace=False)[0]


# revision 62
# speedup vs baseline: 1.0165x; 1.0165x over previous
"""Trainium2 Bass kernel for GQA attention with RoPE (dense transformer).

Problem: B=2, S=2048, H=2048, 16 query heads / 4 KV heads, head_dim 128,
causal flash-style attention, fused QKV + o_proj.

Sharding (8 cores): (batch, head-group) grid. Core c handles batch c//4 and
head group c%4 (4 query heads + their shared KV head). o_proj is computed as
per-group partials reduced on host (tensor-parallel o_proj input split).

v3 vs v2 (245us): warmup trimmed to 6 (real matmuls continue the HAM clock
ramp with useful work), ones matrix via memset, q-projection chains emitted
before k/v so the A psum banks free up early, attention(0) moved to t=1,
v transposes emitted after attention, pairwise ic-outer o_proj tail with
split output DMAs.

On-core layout: activations live as [feature, token] ("transposed") so the
feature contraction dims land on SBUF partitions for the PE array.
Causal masking: fully-masked k-tiles are skipped entirely; diagonal tiles
get a zero-fill triangle (affine_select on GpSimd) after exp.
"""
import math

import numpy as np

import concourse.bass as bass
import concourse.mybir as mybir
import concourse.tile as tile
from concourse import bacc
from concourse.bass_utils import run_bass_kernel_spmd
from concourse.masks import make_identity

B, S, H = 2, 2048, 2048
NH, KVH, HD = 16, 4, 128
G = 4                 # head groups (= KVH); grid = G x B = 8 cores
GQ = NH // KVH        # query heads per group
QD = GQ * HD          # per-core q dim (512)
KC = H // 128         # contraction chunks for projections (16)
TC = 4                # token chunks of 512
TT = S // 128         # 128-token tiles (16)

F32 = mybir.dt.float32
BF = mybir.dt.bfloat16
AF = mybir.ActivationFunctionType

_NC = None


def _emit(nc):
    # All big inputs are host-packed partition-major: row p is the full
    # contiguous per-partition payload, so every DMA is 128 descriptors of
    # >=4KB regardless of logical shape (HWDGE issue cost is ~5ns/descriptor).
    # x is packed token-chunk-major: [p, t, ko, c] so chunk t's whole
    # projection input is one contiguous 16KB-per-partition run.
    xP = nc.dram_tensor("xP", [128, KC * S], BF, kind="ExternalInput").ap()
    wqP = nc.dram_tensor("wqP", [128, KC * QD], BF, kind="ExternalInput").ap()
    wkP = nc.dram_tensor("wkP", [128, KC * HD], BF, kind="ExternalInput").ap()
    wvP = nc.dram_tensor("wvP", [128, KC * HD], BF, kind="ExternalInput").ap()
    woP = nc.dram_tensor("woP", [128, GQ * H], BF, kind="ExternalInput").ap()
    cosT = nc.dram_tensor("cosT", [HD, S], BF, kind="ExternalInput").ap()
    sinS = nc.dram_tensor("sinS", [HD, S], BF, kind="ExternalInput").ap()
    bqkv = nc.dram_tensor("bqkv", [128, 6], F32, kind="ExternalInput").ap()
    outp = nc.dram_tensor("outp", [S, H], BF, kind="ExternalOutput").ap()

    with tile.TileContext(nc) as tc:
        with (
            tc.tile_pool(name="persist", bufs=1) as pp,
            tc.tile_pool(name="qfp", bufs=2) as pqf,
            tc.tile_pool(name="cd", bufs=1) as pd,
            tc.tile_pool(name="expp", bufs=1) as pe,
            tc.tile_pool(name="psum8", bufs=1, space="PSUM") as ps8,
        ):
            # persistent per-chunk K/V (split per t-chunk to keep dep ranges
            # disjoint between the producing chunk and attention readers)
            kf = [pp.tile([128, 512], BF, name=f"kf{t}") for t in range(TC)]
            v_sb = [pp.tile([128, 4, HD], BF, name=f"vsb{t}")
                    for t in range(TC)]
            ofl = pd.tile([128, GQ, S], BF)       # normalized attn outT

            # ---- constants ----
            # ones first (one cheap memset): the PE warmup matmuls run on
            # it, so they start as soon as the GpSimd queue opens instead
            # of waiting for the multi-op make_identity chain.
            ones_mat = pp.tile([128, 128], BF)
            nc.gpsimd.memset(ones_mat[:, :], 1.0)

            # PE warmup: a few dummy matmuls start the HAM activity
            # monitor ramp during the DMA window; the first real matmuls
            # then continue it with useful work (they run at 1.2GHz for
            # the first ~4us either way — better them than more dummies).
            warm = ps8.tile([128, 128], F32, tag="Ct", bufs=2, name="warm")
            for _ in range(6):
                nc.tensor.matmul(warm[:, :], ones_mat[:, :], ones_mat[:, :],
                                 start=True, stop=True)

            ident = pp.tile([128, 128], BF)
            make_identity(nc, ident[:, :])

            bias_sb = pp.tile([128, 6], F32)
            nc.gpsimd.dma_start(bias_sb[:, :], bqkv)

            def jspan(qc, j):
                if j < 4 * qc:
                    q0, n = 512 * qc, 512
                else:
                    q0 = 128 * j
                    n = 512 * (qc + 1) - q0
                return q0, n, q0 - 512 * qc

            def attention(qc, qf_t, filler=None):
                """flash attention for q-chunk qc over k-tiles 0..4qc+3.

                filler(h) emits PE-dense side work (o_proj tiles of the
                previous chunk) interleaved per head, so the scheduler has
                matmuls to run while exp paces the score pipeline."""
                qs = slice(512 * qc, 512 * qc + 512)
                nj = 4 * qc + 4
                state = {}

                def finish(h):
                    # softmax denominator + normalization for head h; emitted
                    # one head late so its ones-matmul never stalls PE on the
                    # DVE accumulation chain.
                    exs, p_o = state[h]
                    p_sum = ps8.tile([128, 512], F32, tag="Bt", bufs=2,
                                     name=f"psum_{h}_{qc}")
                    nc.tensor.matmul(p_sum[:, :], ones_mat[:, :], exs[:, :],
                                     start=True, stop=True)
                    bc = pe.tile([128, 512], F32, tag="bc", bufs=2,
                                 name=f"bc_{h}_{qc}")
                    nc.vector.reciprocal_approx_fast(bc[:, :], p_sum[:, :])
                    nc.vector.tensor_mul(ofl[:, h, qs], p_o[:, :], bc[:, :])

                for h in range(GQ):
                    if filler is not None:
                        filler(h)
                    exs = pe.tile([128, 512], BF, tag="exs", bufs=3,
                                  name=f"exs_{h}_{qc}")
                    exts = []
                    for j in range(nj):
                        q0, n, off = jspan(qc, j)
                        ql = q0 - 512 * qc
                        # chunk 0's 4 score tiles fit in A0/A1 (exp paces
                        # the rotation anyway); A2/A3 then carry its k/v
                        # projection filler chains
                        ps = ps8.tile([128, 512], F32,
                                      tag=f"A{j % 4 if qc else j % 2}",
                                      name=f"ps_{h}_{qc}_{j}")
                        nc.tensor.matmul(
                            ps[:, 0:n], kf[j // 4][:, 128 * (j % 4):
                                                   128 * (j % 4) + 128],
                            qf_t[:, h, ql:ql + n], start=True, stop=True)
                        ex = pe.tile([128, 512], BF, tag="E", bufs=28,
                                     name=f"ex_{h}_{qc}_{j}")
                        nc.scalar.activation(ex[:, 0:n], ps[:, 0:n], AF.Exp)
                        if j >= 4 * qc:
                            # zero the strictly-lower (q < k) triangle
                            nc.gpsimd.affine_select(
                                out=ex[:, 0:128], in_=ex[:, 0:128],
                                compare_op=mybir.AluOpType.is_ge, fill=0.0,
                                base=0, pattern=[[1, 128]],
                                channel_multiplier=-1)
                        if j == 0:
                            nc.vector.tensor_copy(exs[:, :], ex[:, :])
                        else:
                            nc.vector.tensor_add(exs[:, ql:ql + n],
                                                 exs[:, ql:ql + n],
                                                 ex[:, 0:n])
                        exts.append(ex)
                    p_o = ps8.tile([128, 512], F32, tag="Ct", bufs=2,
                                   name=f"po_{h}_{qc}")
                    state[h] = (exs, p_o)
                    for j in range(nj):
                        q0, n, off = jspan(qc, j)
                        nc.tensor.matmul(
                            p_o[:, off:off + n],
                            v_sb[j // 4][:, j % 4, :],
                            exts[j][:, 0:n], start=(j == 0), stop=(j == nj - 1))
                    if h > 0:
                        finish(h - 1)
                finish(GQ - 1)

            def oproj_tile(tt, pwo, wo_sb):
                """o_proj partial for one 128-token tile (filler mode).

                Runs two waves of 2 output-column groups on the Bt psum
                slots only, so the attention pipeline keeps both Ct slots
                for its held p_o accumulators."""
                tsl = slice(128 * tt, 128 * tt + 128)
                fo = pwo.tile([128, 4, 512], BF, tag="fo", bufs=3,
                              name=f"fo_{tt}")
                oc = 0
                for w in range(2):
                    pfs = [ps8.tile([128, 512], F32, tag="Bt", bufs=2,
                                    name=f"pf_{tt}_{oc + i}")
                           for i in range(2)]
                    for ic in range(GQ):
                        for i in range(2):
                            osl = slice(512 * (oc + i), 512 * (oc + i) + 512)
                            nc.tensor.matmul(
                                pfs[i][:, :], ofl[:, ic, tsl],
                                wo_sb[:, ic, osl],
                                start=(ic == 0), stop=(ic == GQ - 1))
                    # split psum evictions between ACT and DVE
                    for i in range(2):
                        if i % 2 == 0:
                            nc.scalar.copy(fo[:, oc + i, :], pfs[i][:, :])
                        else:
                            nc.vector.tensor_copy(fo[:, oc + i, :],
                                                  pfs[i][:, :])
                    oc += 2
                nc.sync.dma_start(outp[tsl, :], fo[:, :, :])

            def oproj_tail(pwo, wo_sb):
                """Final 4 o_proj tiles, after the last attention chunk.

                Tile pairs share a 4-psum wave (A banks, idle now) with the
                head (ic) loop OUTER, so the ic<3 matmuls are not gated on
                the last head's softmax-normalize chain. Each tile's output
                ships as two half-row DMAs on alternating queues so the
                exit barrier waits only on the last 0.25MB."""
                t0 = 4 * (TC - 1)
                fos = {}
                for p in range(2):
                    pair = (t0 + 2 * p, t0 + 2 * p + 1)
                    for tt in pair:
                        fos[tt] = pwo.tile([128, 4, 512], BF, tag="fo",
                                           bufs=3, name=f"fo_{tt}")
                    for w in range(2):
                        combos = [(tt, 2 * w + g) for tt in pair
                                  for g in range(2)]
                        pfs = [ps8.tile([128, 512], F32, tag=f"A{i}",
                                        name=f"pf_{tt}_{oc}")
                               for i, (tt, oc) in enumerate(combos)]
                        for ic in range(GQ):
                            for i, (tt, oc) in enumerate(combos):
                                nc.tensor.matmul(
                                    pfs[i][:, :],
                                    ofl[:, ic, 128 * tt:128 * tt + 128],
                                    wo_sb[:, ic, 512 * oc:512 * oc + 512],
                                    start=(ic == 0), stop=(ic == GQ - 1))
                        for i, (tt, oc) in enumerate(combos):
                            if i % 2 == 0:
                                nc.scalar.copy(fos[tt][:, oc, :], pfs[i][:, :])
                            else:
                                nc.vector.tensor_copy(fos[tt][:, oc, :],
                                                      pfs[i][:, :])
                            # ship each 0.125MB quarter as it lands, on
                            # alternating queues, so the exit barrier waits
                            # only on the very last quarter's transfer
                            eng = [nc.sync, nc.gpsimd][(2 * tt + oc) % 2]
                            eng.dma_start(
                                outp[128 * tt:128 * tt + 128,
                                     512 * oc:512 * oc + 512],
                                fos[tt][:, oc, :])

            # ============ interleaved projections + attention =============
            qf_tiles = [None] * TC
            with (
                tc.tile_pool(name="projw", bufs=1) as pw,
                tc.tile_pool(name="rope", bufs=1) as pr,
                tc.tile_pool(name="wop", bufs=1) as pwo,
            ):
                wq_sb = pw.tile([128, KC, QD], BF)
                wk_sb = pw.tile([128, KC, HD], BF)
                wv_sb = pw.tile([128, KC, HD], BF)
                cos_sb = pw.tile([128, S], BF)
                sin_sb = pw.tile([128, S], BF)
                wo_sb = pwo.tile([128, GQ, H], BF)

                # All input DMAs on the SP HWDGE queue, in first-needed
                # order, sized so the ko=0 accumulation starts after ~1MB.
                x_sb = [None] * TC

                def ld_x(t, pieces):
                    xt = x_sb[t]
                    if xt is None:
                        xt = pw.tile([128, KC, 512], BF, tag="xc", bufs=2,
                                     name=f"x_sb{t}")
                        x_sb[t] = xt
                    for k0, k1 in pieces:
                        nc.sync.dma_start(
                            xt[:, k0:k1, :],
                            xP[:, 512 * (KC * t + k0):512 * (KC * t + k1)])

                def ld_wq(k0, k1):
                    nc.sync.dma_start(wq_sb[:, k0:k1, :],
                                      wqP[:, QD * k0:QD * k1])

                ld_x(0, [(0, 1)])
                ld_wq(0, 1)
                ld_x(0, [(1, 2)])
                ld_wq(1, 2)
                ld_x(0, [(2, 4)])
                ld_wq(2, 4)
                nc.sync.dma_start(wk_sb[:, :, :], wkP)
                nc.sync.dma_start(wv_sb[:, :, :], wvP)
                ld_x(0, [(4, 8)])
                ld_wq(4, 8)
                ld_x(0, [(8, 16)])
                ld_wq(8, 16)
                nc.sync.dma_start(cos_sb[:, :], cosT)
                nc.sync.dma_start(sin_sb[:, :], sinS)

                for t in range(TC):
                    ts = slice(512 * t, 512 * t + 512)
                    if t + 1 < TC:
                        ld_x(t + 1, [(0, 8), (8, 16)])
                    if t == 0:
                        nc.sync.dma_start(wo_sb[:, :, :], woP)

                    pq = [ps8.tile([128, 512], F32, tag=f"A{m}",
                                   name=f"pq{m}_{t}")
                          for m in range(GQ)]
                    # q first over all ko, then k, then v: the A psum banks
                    # finish (and can be evicted) while the k/v chains still
                    # run, so the attention scores below wait less.
                    for ko in range(KC):
                        st = (ko == 0)
                        sp = (ko == KC - 1)
                        xc = x_sb[t][:, ko, :]
                        for m in range(GQ):
                            nc.tensor.matmul(
                                pq[m][:, :],
                                wq_sb[:, ko, 128 * m:128 * m + 128],
                                xc, start=st, stop=sp)

                    # q psum evictions (+bias) on ACT, one by one as each
                    # head's chain retires
                    raws = []
                    for m in range(GQ):
                        raw = pr.tile([128, 512], BF, tag="raw", bufs=6,
                                      name=f"raw_{t}_{m}")
                        nc.scalar.activation(
                            raw[:, :], pq[m][:, :], AF.Identity,
                            bias=bias_sb[:, m:m + 1])
                        raws.append((m, raw))

                    vT_box = [None]

                    def emit_k(t=t, raws=raws, tag="Bt", bufs=2):
                        pk = ps8.tile([128, 512], F32, tag=tag, bufs=bufs,
                                      name=f"pk_{t}")
                        for ko in range(KC):
                            nc.tensor.matmul(
                                pk[:, :], wk_sb[:, ko, :],
                                x_sb[t][:, ko, :],
                                start=(ko == 0), stop=(ko == KC - 1))
                        rawk = pr.tile([128, 512], BF, tag="raw", bufs=6,
                                       name=f"raw_{t}_k")
                        nc.scalar.activation(rawk[:, :], pk[:, :],
                                             AF.Identity,
                                             bias=bias_sb[:, 4:5])
                        raws.append((GQ, rawk))

                    def emit_v(t=t, vT_box=vT_box, tag="Bt", bufs=2):
                        pv = ps8.tile([128, 512], F32, tag=tag, bufs=bufs,
                                      name=f"pv_{t}")
                        for ko in range(KC):
                            nc.tensor.matmul(
                                pv[:, :], wv_sb[:, ko, :],
                                x_sb[t][:, ko, :],
                                start=(ko == 0), stop=(ko == KC - 1))
                        # v: evict with bias (frees the psum bank before
                        # attention needs it for softmax denominators)
                        vT_t = pr.tile([128, 512], BF, tag="vT", bufs=2,
                                       name=f"vT_{t}")
                        nc.scalar.activation(vT_t[:, :], pv[:, :],
                                             AF.Identity,
                                             bias=bias_sb[:, 5:6])
                        vT_box[0] = vT_t

                    # attention + o_proj for the previous chunk, emitted
                    # before this chunk's v-transpose and RoPE (both only
                    # feed the NEXT iteration) so the PE never waits on
                    # their eviction chains. attention(0) has no o_proj of
                    # its own to interleave — its PE filler is this chunk's
                    # k/v chains (on the A2/A3 banks its scores don't use).
                    if t == 1:
                        fill = {0: lambda: emit_k(tag="A2", bufs=1),
                                1: lambda: emit_v(tag="A3", bufs=1)}
                        attention(0, qf_tiles[0],
                                  filler=lambda h: fill.get(h, lambda: 0)())
                    else:
                        emit_k()
                        emit_v()
                        if t >= 2:
                            qc = t - 1
                            attention(qc, qf_tiles[qc],
                                      filler=lambda h, q=qc: oproj_tile(
                                          4 * (q - 1) + h, pwo, wo_sb))

                    # v transpose to natural [tok, d] layout (needed by the
                    # NEXT iteration's attention)
                    vT_t = vT_box[0]
                    for st4 in range(4):
                        ptr = ps8.tile([128, 128], BF, tag="Ct", bufs=2,
                                       name=f"ptr_{t}_{st4}")
                        nc.tensor.transpose(
                            ptr[:, :], vT_t[:, 128 * st4:128 * st4 + 128],
                            ident[:, :])
                        nc.scalar.copy(v_sb[t][:, st4, :], ptr[:, :])

                    # RoPE for q/k on DVE in [d, tok] layout (needed only by
                    # THIS chunk's attention, one iteration later)
                    qf_t = pqf.tile([128, GQ, 512], BF, tag="qf",
                                    name=f"qf_{t}")
                    qf_tiles[t] = qf_t
                    for m, raw in raws:
                        rot = pr.tile([128, 512], BF, tag="rot", bufs=2,
                                      name=f"rot_{t}_{m}")
                        nc.vector.tensor_copy(rot[0:64, :], raw[64:128, :])
                        nc.vector.tensor_copy(rot[64:128, :], raw[0:64, :])
                        t1 = pr.tile([128, 512], BF, tag="t1", bufs=2,
                                     name=f"t1_{t}_{m}")
                        nc.vector.tensor_mul(t1[:, :], rot[:, :],
                                             sin_sb[:, ts])
                        t2 = pr.tile([128, 512], BF, tag="t2", bufs=2,
                                     name=f"t2_{t}_{m}")
                        nc.vector.tensor_mul(t2[:, :], raw[:, :],
                                             cos_sb[:, ts])
                        dst = (qf_t[:, m, :] if m < GQ else kf[t][:, :])
                        nc.vector.tensor_add(dst, t1[:, :], t2[:, :])
                qc = TC - 1
                attention(qc, qf_tiles[qc],
                          filler=lambda h: oproj_tile(4 * (qc - 1) + h,
                                                      pwo, wo_sb))
                oproj_tail(pwo, wo_sb)


def _build():
    global _NC
    if _NC is None:
        nc = bacc.Bacc("TRN2", target_bir_lowering=False, debug=False,
                       num_devices=8)
        _emit(nc)
        nc.compile()
        _NC = nc
    return _NC


def _prep_inputs(x, wq, bq, wk, bk, wv, bv, wo, bo, cos, sin):
    """Host-side shard + layout prep. Core c = (g, b): g = c % 4, b = c // 4."""
    import ml_dtypes
    bf16 = ml_dtypes.bfloat16
    inv_sqrt_d = 1.0 / math.sqrt(HD)
    f32 = np.float32
    cosT = np.ascontiguousarray(cos.T.astype(bf16))
    sinSf = sin.T.astype(f32).copy()
    sinSf[0:HD // 2] *= -1.0
    sinS = np.ascontiguousarray(sinSf.astype(bf16))

    def pack(mT):
        """[n*128, m] -> [128, n*m]: row p = concat_n mT[n*128+p, :]."""
        n = mT.shape[0] // 128
        return np.ascontiguousarray(
            mT.reshape(n, 128, mT.shape[1]).transpose(1, 0, 2)
            .reshape(128, n * mT.shape[1]).astype(bf16))

    def pack_x(xT):
        """[H, S] -> [128, TC*KC*512]: [p, t, ko, c] = xT[ko*128+p, 512t+c]
        (token-chunk-major so each projection chunk is one contiguous run)."""
        return np.ascontiguousarray(
            xT.reshape(KC, 128, TC, 512).transpose(1, 2, 0, 3)
            .reshape(128, TC * KC * 512).astype(bf16))

    xPb = [pack_x(x[b].T.astype(f32)) for b in range(B)]

    in_maps = []
    for c in range(8):
        g, b = c % G, c // G
        wq_s = wq[QD * g:QD * (g + 1), :] * inv_sqrt_d
        bq_s = bq[QD * g:QD * (g + 1)] * inv_sqrt_d
        wk_s = wk[HD * g:HD * (g + 1), :]
        bk_s = bk[HD * g:HD * (g + 1)]
        wv_s = wv[HD * g:HD * (g + 1), :]
        bv_s = bv[HD * g:HD * (g + 1)]
        bias = np.zeros((128, 6), f32)
        bias[:, 0:4] = bq_s.reshape(GQ, HD).T
        bias[:, 4] = bk_s
        bias[:, 5] = bv_s
        in_maps.append({
            "xP": xPb[b],
            "wqP": pack(wq_s.T),
            "wkP": pack(wk_s.T),
            "wvP": pack(wv_s.T),
            "woP": pack(wo[:, QD * g:QD * (g + 1)].T),
            "cosT": cosT,
            "sinS": sinS,
            "bqkv": bias,
        })
    return in_maps


def run(inputs, trace=False):
    """Returns (full_output, BassKernelResults)."""
    inputs = {k: np.asarray(v) for k, v in inputs.items()}
    nc = _build()
    in_maps = _prep_inputs(**inputs)
    res = run_bass_kernel_spmd(nc, in_maps, core_ids=list(range(8)),
                               trace=trace)
    bo = inputs["bo"].astype(np.float64)
    out = np.empty((B, S, H), np.float32)
    for b in range(B):
        acc = np.zeros((S, H), np.float64)
        for g in range(G):
            acc += res.results[G * b + g]["outp"].astype(np.float64)
        out[b] = (acc + bo).astype(np.float32)
    return out, res


def kernel(**inputs):
    return run(inputs, trace=False)[0]
